# revision 24
# baseline (speedup 1.0000x reference)
"""Trainium2 Bass kernel for nn_AttentionBlock (S=4096, H=1024, NH=2, DS=64).

Strategy (v2): full sequence parallelism. Each core owns 512 rows (queries
AND keys): it computes Q^T, K^T, V for its own block only (bf16 operands),
then the K^T/V shards are exchanged with 4 chunked AllGathers (one per
key-quarter) so attention overlaps the collectives. Full K^T and V stay
SBUF-resident in bf16 (64KB + 64KB per partition). Heads are interleaved
per key-quarter; ctx partial sums accumulate in SBUF f32 (PSUM holds the
two lsum rows for the whole sweep). Out-projection + residual + LayerNorm
as in v1.
"""

import math
import sys

sys.path.insert(0, "/opt/trn_rl_repo")

import numpy as np
import ml_dtypes

import concourse.bass as bass
import concourse.mybir as mybir
import concourse.tile as tile
from concourse import bacc
from concourse.bass_utils import run_bass_kernel_spmd

S, H, NH, DS = 4096, 1024, 2, 64
HD = H // NH            # 512
NC = 8                  # cores
SQ = S // NC            # 512 queries (and keys) per core
EPS = 1e-5
F32 = mybir.dt.float32
F32R = mybir.dt.float32r
BF16 = mybir.dt.bfloat16
AF = mybir.ActivationFunctionType
ALU = mybir.AluOpType

KC = S // 128           # 32 key chunks of 128 (global)
HC = H // 128           # 8 hidden chunks of 128
QB = SQ // 128          # 4 query chunks of 128
NJ = 4                  # key sub-chunks per core block (AG quarters)
AGW = HC * 128 + H      # 2048 bf16 elems per partition per AG quarter


def build_program():
    nc = bacc.Bacc("TRN2", target_bir_lowering=False, debug=False, num_devices=NC)

    # ---- DRAM I/O ----
    xq = nc.dram_tensor("xq", [SQ, H], F32, kind="ExternalInput")
    xqb = nc.dram_tensor("xqb", [SQ, H], BF16, kind="ExternalInput")
    wqT = nc.dram_tensor("wqT", [H, H], BF16, kind="ExternalInput")
    wkT = nc.dram_tensor("wkT", [H, H], BF16, kind="ExternalInput")
    wvT = nc.dram_tensor("wvT", [H, H], BF16, kind="ExternalInput")
    woT = nc.dram_tensor("woT", [H, H], BF16, kind="ExternalInput")
    wsT = nc.dram_tensor("wsT", [DS, H], F32R, kind="ExternalInput")
    sdat = nc.dram_tensor("sdat", [DS, 1], F32R, kind="ExternalInput")
    bsv = nc.dram_tensor("bsv", [H], F32, kind="ExternalInput")
    mbias = nc.dram_tensor("mbias", [128, KC], F32, kind="ExternalInput")
    onescol = nc.dram_tensor("onescol", [128, 1], BF16, kind="ExternalInput")
    onesrow = nc.dram_tensor("onesrow", [1, 128], BF16, kind="ExternalInput")
    identd = nc.dram_tensor("identd", [128, 128], BF16, kind="ExternalInput")
    lnw = nc.dram_tensor("lnw", [H], F32, kind="ExternalInput")
    lnb = nc.dram_tensor("lnb", [H], F32, kind="ExternalInput")
    out = nc.dram_tensor("out", [SQ, H], F32, kind="ExternalOutput")

    inv_sqrt_hd = 1.0 / math.sqrt(HD)

    with tile.TileContext(nc) as tc:
        with (
            tc.tile_pool(name="consts", bufs=1) as consts,
            tc.tile_pool(name="persist", bufs=1) as persist,
            tc.tile_pool(name="rlp", bufs=1) as rlp,
            tc.tile_pool(name="dram", bufs=1, space="DRAM") as dram,
        ):
            # ---- constants; sync queue carries only the stage-1-critical
            # ones (semb chain + transpose ident + x rows), the rest ride
            # the gpsimd queue which is otherwise idle until the collectives.
            sd_sb = consts.tile([DS, 1], F32R)
            nc.sync.dma_start(sd_sb, sdat[:, :])
            wsT_sb = consts.tile([DS, H], F32R)
            nc.sync.dma_start(wsT_sb, wsT[:, :])
            ident = consts.tile([128, 128], BF16)
            nc.sync.dma_start(ident, identd[:, :])
            Af = consts.tile([128, 36], F32)     # 0:32 maskbias | 32 zero | 33 eps
            mb_sb = Af[:, 0:32]
            nc.gpsimd.dma_start(mb_sb, mbias[:, :])
            zb_sb = Af[:, 32:33]
            nc.vector.memset(zb_sb, 0.0)
            eps_sb = Af[:, 33:34]
            nc.vector.memset(eps_sb, EPS)
            ones_sb = consts.tile([128, 1], BF16)
            nc.gpsimd.dma_start(ones_sb, onescol[:, :])
            onesrow_sb = consts.tile([1, 128], BF16)
            nc.gpsimd.dma_start(onesrow_sb, onesrow[:, :])
            lnw_b = consts.tile([128, H], F32)
            nc.gpsimd.dma_start(lnw_b, bass.AP(tensor=lnw, offset=0, ap=[[0, 128], [1, H]]))
            lnb_b = consts.tile([128, H], F32)
            nc.gpsimd.dma_start(lnb_b, bass.AP(tensor=lnb, offset=0, ap=[[0, 128], [1, H]]))

            # ---- persistent tiles ----
            qT_sb = persist.tile([128, HC, SQ], BF16)      # Q^T/sqrt(hd): [d, q]
            ctxT = persist.tile([128, HC, SQ], BF16)       # ctx^T/l: [d, q]
            wo_sb = persist.tile([128, HC, H], BF16)
            vb_bcast = persist.tile([128, H], F32)
            semb_bf = persist.tile([128, HC], BF16)

            # DRAM scratch
            semb_scr = dram.tile([H], F32)
            vb_scr = dram.tile([H], F32)
            kvin = dram.tile([NJ, 128, AGW], BF16)
            agouts = [dram.tile([NC, 128, AGW], BF16, addr_space="Shared",
                                name=f"agout{j}")
                      for j in range(NJ)]

            # warm-up collective: absorbs the cross-core rendezvous/launch
            # skew while stage 1 computes, so the first real AllGather runs
            # at steady state
            warm_in = dram.tile([1, 32], F32)
            warm_out = dram.tile([NC, 1, 32], F32, addr_space="Shared")
            nc.gpsimd.dma_start(warm_in[:], mbias[0:1, 0:32])
            nc.gpsimd.collective_compute(
                "AllGather", mybir.AluOpType.bypass,
                replica_groups=[list(range(NC))],
                ins=[warm_in.opt()], outs=[warm_out.opt()],
            )

            # ================= Stage 1: own-block projections ==================
            # Order targets the critical path to the first AllGather: K^T and
            # V shards first (ship each key-quarter as soon as ready), Q^T
            # last (it overlaps AllGather 0).
            with (
                tc.tile_pool(name="s1", bufs=1) as s1,
                tc.tile_pool(name="xtp", bufs=2) as xtp,
                tc.tile_pool(name="ps1", bufs=3, space="PSUM") as ps1,
                tc.tile_pool(name="pst", bufs=2, space="PSUM") as pst,
                tc.tile_pool(name="psb", bufs=2, space="PSUM") as psb,
            ):
                wk_sb = s1.tile([128, HC, H], BF16, tag="wk")
                nc.scalar.dma_start(wk_sb, wkT.rearrange("(c p) d -> p c d", p=128))
                wv_sb = s1.tile([128, HC, H], BF16, tag="wv")
                nc.scalar.dma_start(wv_sb, wvT.rearrange("(c p) d -> p c d", p=128))
                wq_sb = s1.tile([128, HC, H], BF16, tag="wq")
                nc.scalar.dma_start(wq_sb, wqT.rearrange("(c p) d -> p c d", p=128))
                nc.scalar.dma_start(wo_sb, woT.rearrange("(c p) d -> p c d", p=128))
                xT_sb = s1.tile([128, HC, SQ], BF16, tag="xT")
                ktown = s1.tile([128, HC, SQ], BF16, tag="ktown")
                vown = s1.tile([128, NJ, H], BF16, tag="vown")

                # --- transpose own x rows -> x^T (bf16) ---
                for qb in range(QB):
                    xin = xtp.tile([128, H], BF16, tag="xin", bufs=3, name=f"xin{qb}")
                    nc.sync.dma_start(xin[:, 0:512], xqb[qb * 128:(qb + 1) * 128, 0:512])
                    nc.scalar.dma_start(xin[:, 512:H],
                                        xqb[qb * 128:(qb + 1) * 128, 512:H])
                    for hc in range(HC):
                        pt = pst.tile([128, 128], BF16, tag="ptr", name=f"tr{qb}_{hc}")
                        nc.tensor.transpose(
                            pt[:], xin[:, hc * 128:(hc + 1) * 128], ident)
                        nc.any.tensor_copy(xT_sb[:, hc, qb * 128:(qb + 1) * 128],
                                           pt[:])

                # --- semb = Ws @ static + bs; then bias rows (roundtrips) ---
                bs_row = rlp.tile([1, H], F32, tag="row", name="bs_row")
                nc.scalar.dma_start(bs_row, bsv.rearrange("d -> () d"))
                semb_row = rlp.tile([1, H], F32, tag="srow")
                for d2 in range(H // 512):
                    p = psb.tile([1, 512], F32, tag="pbias", name=f"sembp{d2}")
                    nc.tensor.matmul(p[:], sd_sb[:], wsT_sb[:, d2 * 512:(d2 + 1) * 512],
                                     start=True, stop=True)
                    nc.vector.tensor_add(semb_row[:, d2 * 512:(d2 + 1) * 512], p[:],
                                         bs_row[:, d2 * 512:(d2 + 1) * 512])
                nc.scalar.dma_start(semb_scr.rearrange("d -> () d"), semb_row[:])
                semb_pc = rlp.tile([128, HC], F32, tag="spc")
                nc.scalar.dma_start(semb_pc, semb_scr.rearrange("(c p) -> p c", p=128))
                nc.vector.tensor_copy(semb_bf, semb_pc)

                # --- vbias row (kbias is skipped: Wk@semb adds a per-query
                # constant to the logits, which cancels in softmax) ---
                vb_row = rlp.tile([1, H], F32, tag="row", name="vb_row")
                for d2 in range(H // 512):
                    p = psb.tile([1, 512], F32, tag="pbias", name=f"vbp{d2}")
                    for hc in range(HC):
                        nc.tensor.matmul(p[:], semb_bf[:, hc:hc + 1],
                                         wv_sb[:, hc, d2 * 512:(d2 + 1) * 512],
                                         start=(hc == 0), stop=(hc == HC - 1))
                    nc.vector.tensor_copy(vb_row[:, d2 * 512:(d2 + 1) * 512], p[:])
                nc.scalar.dma_start(vb_scr.rearrange("d -> () d"), vb_row[:])
                nc.scalar.dma_start(vb_bcast,
                                    bass.AP(tensor=vb_scr.tensor, offset=vb_scr.offset,
                                            ap=[[0, 128], [1, H]]))

                # --- K^T own (+kbias), then V own (+vbias) per quarter, ship
                # each quarter immediately; Q^T (scaled) last ---
                for dc in range(HC):
                    p = ps1.tile([128, SQ], F32, tag="pproj", name=f"kp{dc}")
                    for hc in range(HC):
                        nc.tensor.matmul(p[:], wk_sb[:, hc, dc * 128:(dc + 1) * 128],
                                         xT_sb[:, hc, :],
                                         start=(hc == 0), stop=(hc == HC - 1))
                    nc.scalar.copy(ktown[:, dc, :], p[:])
                for j in range(NJ):
                    for d2 in range(H // 512):
                        p = ps1.tile([128, 512], F32, tag="pproj", name=f"vp{j}_{d2}")
                        for hc in range(HC):
                            nc.tensor.matmul(p[:], xT_sb[:, hc, j * 128:(j + 1) * 128],
                                             wv_sb[:, hc, d2 * 512:(d2 + 1) * 512],
                                             start=(hc == 0), stop=(hc == HC - 1))
                        nc.vector.tensor_add(vown[:, j, d2 * 512:(d2 + 1) * 512], p[:],
                                             vb_bcast[:, d2 * 512:(d2 + 1) * 512])
                    nc.gpsimd.dma_start(
                        kvin[j, :, 0:HC * 128].rearrange("p (dc k) -> p dc k", dc=HC),
                        ktown[:, :, j * 128:(j + 1) * 128])
                    nc.gpsimd.dma_start(kvin[j, :, HC * 128:AGW], vown[:, j, :])
                for dc in range(HC):
                    p = ps1.tile([128, SQ], F32, tag="pproj", name=f"qp{dc}")
                    for hc in range(HC):
                        nc.tensor.matmul(p[:], wq_sb[:, hc, dc * 128:(dc + 1) * 128],
                                         xT_sb[:, hc, :],
                                         start=(hc == 0), stop=(hc == HC - 1))
                    nc.scalar.mul(qT_sb[:, dc, :], p[:], inv_sqrt_hd)

            # ============ Stage 2: chunked AllGather + attention ==============
            with (
                tc.tile_pool(name="kv", bufs=1) as kv,
                tc.tile_pool(name="attn", bufs=1) as attn,
                tc.tile_pool(name="ps_s", bufs=2, space="PSUM") as ps_s,
                tc.tile_pool(name="ps_c", bufs=1, space="PSUM") as ps_c,
                tc.tile_pool(name="ps_l", bufs=1, space="PSUM") as ps_l,
            ):
                # [p(d), j, c, dc*128+k]: matches the AllGather output layout
                kT_full = kv.tile([128, NJ, NC, HC * 128], BF16, tag="kT")
                v_full = kv.tile([128, KC, H], BF16, tag="v")      # [k, d]
                ctx_acc = kv.tile([128, HC, SQ], F32, tag="ctxa")  # unnormalized

                for j in range(NJ):
                    nc.gpsimd.collective_compute(
                        "AllGather",
                        mybir.AluOpType.bypass,
                        replica_groups=[list(range(NC))],
                        ins=[kvin[j].opt()],
                        outs=[agouts[j].opt()],
                    )
                for j in range(NJ):
                    for c in range(NC):
                        nc.sync.dma_start(
                            kT_full[:, j, c, :],
                            agouts[j][c, :, 0:HC * 128])
                        nc.gpsimd.dma_start(
                            v_full[:, c * NJ + j, :],
                            agouts[j][c, :, HC * 128:AGW])

                lsums = [ps_l.tile([1, SQ], F32, tag=f"l{h}", name=f"lsum{h}")
                         for h in range(NH)]

                def normalize(h):
                    # ctxT_h = ctx_acc_h / l_h (reciprocal broadcast via PE)
                    lrow = rlp.tile([1, SQ], BF16, tag="rl", name=f"lrow{h}")
                    nc.scalar.copy(lrow[:], lsums[h][:])
                    lb_ps = ps_s.tile([128, SQ], F32, tag="st", name=f"lbps{h}")
                    nc.tensor.matmul(lb_ps[:], onesrow_sb[:], lrow[:],
                                     start=True, stop=True)
                    rl_b = rlp.tile([128, SQ], F32, tag="rlb", name=f"rlb{h}")
                    nc.vector.reciprocal(rl_b[:], lb_ps[:])
                    for dv in range(4):
                        nc.vector.tensor_mul(ctxT[:, 4 * h + dv, :],
                                             ctx_acc[:, 4 * h + dv, :], rl_b[:])

                for j in range(NJ):
                    for h in range(NH):
                        ctx_ps = [ps_c.tile([128, SQ], F32, tag=f"ctx{dv}",
                                            name=f"ctxps{j}_{h}_{dv}")
                                  for dv in range(4)]
                        PTs = {}

                        def consume(c, j=j, h=h, PTs=PTs, ctx_ps=ctx_ps):
                            PTk = PTs.pop(c)
                            nc.tensor.matmul(lsums[h][:], ones_sb, PTk[:],
                                             start=(j == 0 and c == 0),
                                             stop=(j == NJ - 1 and c == NC - 1),
                                             skip_group_check=True)
                            kc = c * NJ + j
                            for dv in range(4):
                                nc.tensor.matmul(ctx_ps[dv][:],
                                                 v_full[:, kc,
                                                        (4 * h + dv) * 128:
                                                        (4 * h + dv + 1) * 128],
                                                 PTk[:],
                                                 start=(c == 0), stop=(c == NC - 1),
                                                 skip_group_check=True)

                        for c in range(NC):
                            kc = c * NJ + j     # global 128-key chunk id
                            ps = ps_s.tile([128, SQ], F32, tag="st",
                                           name=f"st{j}_{h}_{c}")
                            for dq in range(4):
                                nc.tensor.matmul(
                                    ps[:],
                                    kT_full[:, j, c,
                                            (4 * h + dq) * 128:(4 * h + dq + 1) * 128],
                                    qT_sb[:, 4 * h + dq, :],
                                    start=(dq == 0), stop=(dq == 3))
                            PTk = attn.tile([128, SQ], BF16, tag="PTs", bufs=3,
                                            name=f"PT{j}_{h}_{c}")
                            PTs[c] = PTk
                            bias_ap = mb_sb[:, kc:kc + 1] if h == 0 else zb_sb
                            nc.scalar.activation(PTk[:], ps[:], AF.Exp, bias=bias_ap)
                            if c > 0:
                                consume(c - 1)
                        consume(NC - 1)

                        # fold this quarter's ctx into the f32 accumulator
                        for dv in range(4):
                            dst = ctx_acc[:, 4 * h + dv, :]
                            if j == 0:
                                nc.vector.tensor_copy(dst, ctx_ps[dv][:])
                            else:
                                nc.vector.tensor_add(dst, dst, ctx_ps[dv][:])
                        if j == NJ - 1:
                            normalize(h)

            # ---- Stage 3: out-proj (natural layout) + residual + LN ----
            with (
                tc.tile_pool(name="s4", bufs=2) as s4,
                tc.tile_pool(name="ps4", bufs=2, space="PSUM") as ps4,
            ):
                for qb in range(QB):
                    xq_f = s4.tile([128, H], F32, tag="xqf", name=f"xqf{qb}")
                    nc.sync.dma_start(xq_f, xq[qb * 128:(qb + 1) * 128, :])
                    res_f = s4.tile([128, H], F32, tag="resf", name=f"resf{qb}")
                    for h2 in range(H // 512):
                        p = ps4.tile([128, 512], F32, tag="pout", name=f"po{qb}_{h2}")
                        for dc in range(HC):
                            nc.tensor.matmul(p[:],
                                             ctxT[:, dc, qb * 128:(qb + 1) * 128],
                                             wo_sb[:, dc, h2 * 512:(h2 + 1) * 512],
                                             start=(dc == 0), stop=(dc == HC - 1))
                        nc.vector.tensor_add(res_f[:, h2 * 512:(h2 + 1) * 512], p[:],
                                             xq_f[:, h2 * 512:(h2 + 1) * 512])
                    # LayerNorm via bn_stats
                    LS = s4.tile([128, 16], F32, tag="lns", name=f"lns{qb}")
                    for h2 in range(H // 512):
                        nc.vector.bn_stats(
                            LS[:, h2 * 6:(h2 + 1) * 6]
                            .rearrange("p (a b) -> p a b", a=1),
                            res_f[:, h2 * 512:(h2 + 1) * 512])
                    nc.vector.bn_aggr(LS[:, 12:14], LS[:, 0:12]
                                      .rearrange("p (a b) -> p a b", a=2))
                    nc.scalar.activation(LS[:, 14:15], LS[:, 13:14], AF.Sqrt,
                                         bias=eps_sb)
                    nc.vector.reciprocal(LS[:, 15:16], LS[:, 14:15])
                    norm = s4.tile([128, H], F32, tag="norm", name=f"norm{qb}", bufs=1)
                    scl = s4.tile([128, H], F32, tag="scl", name=f"scl{qb}", bufs=1)
                    fin = s4.tile([128, H], F32, tag="fin", name=f"fin{qb}")
                    for h2 in range(H // 512):
                        sl = slice(h2 * 512, (h2 + 1) * 512)
                        nc.vector.tensor_scalar(norm[:, sl], res_f[:, sl],
                                                LS[:, 12:13], LS[:, 15:16],
                                                ALU.subtract, ALU.mult)
                        nc.vector.tensor_mul(scl[:, sl], norm[:, sl], lnw_b[:, sl])
                        nc.vector.tensor_add(fin[:, sl], scl[:, sl], lnb_b[:, sl])
                        oeng = nc.sync if qb % 2 == 0 else nc.gpsimd
                        oeng.dma_start(out[qb * 128:(qb + 1) * 128, sl],
                                       fin[:, sl])

    nc.compile()
    return nc


_CACHED_NC = None


def _get_nc():
    global _CACHED_NC
    if _CACHED_NC is None:
        _CACHED_NC = build_program()
    return _CACHED_NC


def _prep_inputs(inputs, static_data, base_mask, Wq, Wk, Wv, Wo, Ws, bs, ln_w, ln_b):
    f32 = np.float32
    bf16 = ml_dtypes.bfloat16
    xf = np.ascontiguousarray(inputs, f32)
    common = {
        "wqT": np.ascontiguousarray(np.asarray(Wq, f32).T).astype(bf16),
        "wkT": np.ascontiguousarray(np.asarray(Wk, f32).T).astype(bf16),
        "wvT": np.ascontiguousarray(np.asarray(Wv, f32).T).astype(bf16),
        "woT": np.ascontiguousarray(np.asarray(Wo, f32).T).astype(bf16),
        "wsT": np.ascontiguousarray(np.asarray(Ws, f32).T),
        "sdat": np.ascontiguousarray(np.asarray(static_data, f32).reshape(DS, 1)),
        "bsv": np.ascontiguousarray(bs, f32),
        "mbias": np.ascontiguousarray(
            np.where(np.asarray(base_mask, bool), 0.0, -1e30)
            .astype(f32).reshape(KC, 128).T),
        "onescol": np.ones((128, 1), bf16),
        "onesrow": np.ones((1, 128), bf16),
        "identd": np.eye(128, dtype=f32).astype(bf16),
        "lnw": np.ascontiguousarray(ln_w, f32),
        "lnb": np.ascontiguousarray(ln_b, f32),
    }
    in_maps = []
    for c in range(NC):
        m = dict(common)
        m["xq"] = np.ascontiguousarray(xf[c * SQ:(c + 1) * SQ, :])
        m["xqb"] = np.ascontiguousarray(xf[c * SQ:(c + 1) * SQ, :]).astype(bf16)
        in_maps.append(m)
    return in_maps


def kernel_run(trace=False, **inputs):
    nc = _get_nc()
    in_maps = _prep_inputs(**inputs)
    res = run_bass_kernel_spmd(nc, in_maps, core_ids=list(range(NC)), trace=trace)
    outp = np.concatenate([res.results[c]["out"] for c in range(NC)], axis=0)
    return outp, res


def kernel(**inputs):
    outp, _ = kernel_run(trace=False, **inputs)
    return outp
